# revision 38
# baseline (speedup 1.0000x reference)
"""Trainium2 Bass kernel for nn_AVDFullLinearMix.

Math (folded form, all terms single matmuls over raw inputs):
  x_d_out = x_d + W_ttrans @ x_d^T(spatial) + (W_tdelta @ x_a) * eye3
            + eps_expand(W_vd @ x_v)
  x_a_out = x_a + W_ct @ trace(x_d),        W_ct = W_ttrace @ (I + W_ttrans)
  x_v_out = x_v + W_cd @ eps_contract(x_d), W_cd = W_dv @ (I - W_ttrans)
(the TensDelta term never reaches the eps-contract since eps[i,i,k] = 0,
 and trace/eps-contract commute with the TensTrans spatial transpose up to
 identity/negation, so both weight chains fold on the host.)

The device computes and returns only the DELTAS in fp16; the exact-fp32
residual add (x + delta) happens on the host. This halves both stream
directions (inputs also stream fp16 since every consumer goes through the
fp16/fp22 matmul datapath anyway).

Sharding: data-parallel over 8 cores (2048 tokens each); weights replicated.
Device layout is channel-major and chunk-packed: one input tensor
[256, chunk, 13, tok] per core holding (9 x_d planes, 3 x_v planes, 1 x_a).
The sync HWDGE ring carries everything with a 3-chunk in-DMA lookahead so
the FIFO never head-of-line blocks, and in/out transfers alternate so HBM
reads and writes overlap. The shared delta (W_tdelta @ x_a) and the three
vd products (W_vd @ x_v) are computed once per chunk and reused across the
diagonal / off-diagonal planes via the DVE combine (add/sub handles the
Levi-Civita sign); pure PSUM->SBUF copies ride the otherwise-idle Scalar
engine so VectorE only does the y/tr builds and the 9 ttrans combines.
"""

import numpy as np

import concourse.bass as bass  # noqa: F401  (registers handle types)
import concourse.mybir as mybir
import concourse.tile as tile
from concourse import bacc
from concourse.bass_utils import run_bass_kernel_spmd

NCORES = 8
P = 128          # partitions
C = 256          # channels
B, N = 16, 1024
T = (B * N) // NCORES   # tokens per core = 2048
CHUNKS = [256] * 7 + [128, 128]   # token count per chunk (sum = T); small
OFFS = [0]                         # final chunks shorten the compute tail
for _tch in CHUNKS[:-1]:
    OFFS.append(OFFS[-1] + _tch)
NCHUNK = len(CHUNKS)
NPL = 13                # planes per chunk: 9 x_d, 3 x_v, 1 x_a
PL_XV = 9
PL_XA = 12
OUT_SPLIT = 7           # out-DMA part A = planes [0:7), part B = [7:13)

F32 = mybir.dt.float32
F16 = mybir.dt.float16

# flat spatial index s = i*3 + j
SPERM = [0, 3, 6, 1, 4, 7, 2, 5, 8]    # s -> transposed flat (j*3+i)
DIAG = (0, 4, 8)
# off-diag s=(k,j): x_d_out[...,k,j] += eps[i*,j,k] * (W_vd @ x_v[..,i*])
# VD_MAP: s -> (i*, sign)
VD_MAP = {1: (2, -1), 2: (1, +1), 3: (2, +1), 5: (0, -1), 6: (1, -1), 7: (0, +1)}
# eps-contract y[:,k] = x_d[:,s1] - x_d[:,s2]
Y_PAIRS = [(5, 7), (6, 2), (1, 3)]
# weight slots (lhsT layout W.T): 0=ttrans 1=tdelta 2=vd 3=cd 4=ct
NW = 5

_CACHE: dict = {}


def _build():
    nc = bacc.Bacc(None, target_bir_lowering=False)
    xin = nc.dram_tensor("xin", [C, NPL * T], F16, kind="ExternalInput")
    wts = nc.dram_tensor("wts", [NW, C, C], F16, kind="ExternalInput")
    xout = nc.dram_tensor("xout", [C, NPL * T], F16, kind="ExternalOutput")

    # channel-split dram views: c = h*128 + p
    xin_hp = xin.rearrange("(h p) f -> p h f", p=P)
    xout_hp = xout.rearrange("(h p) f -> p h f", p=P)

    def dram_ck(base, ck, lo=0, hi=NPL):
        tch = CHUNKS[ck]
        off = NPL * OFFS[ck]
        v = base[:, :, off + lo * tch: off + hi * tch]
        return v.rearrange("p h (s t) -> p h s t", t=tch)
    wts_r = wts.rearrange("w (kh p) o -> p w kh o", p=P)

    with tile.TileContext(nc) as tc:
        with (
            tc.tile_pool(name="wpool", bufs=1) as wpool,
            tc.tile_pool(name="data", bufs=3) as data,
            tc.tile_pool(name="outp", bufs=3) as outp,
            tc.tile_pool(name="tmp", bufs=2) as tmp,
            tc.tile_pool(name="psum", bufs=4, space="PSUM") as psum,
        ):
            w_sb = wpool.tile([P, NW, 2, C], F16)
            nc.sync.dma_start(w_sb[:, 1:3], wts_r[:, 1:3])
            nc.sync.dma_start(w_sb[:, 0], wts_r[:, 0])
            nc.sync.dma_start(w_sb[:, 3:], wts_r[:, 3:])

            # HAM warm-up: ~6us of discarded matmuls while the first input
            # DMA is in flight, so chunk 0 runs at 2.4 GHz instead of 1.2
            scratch = wpool.tile([P, 2, 256], F16)
            scratch_ps = psum.tile([P, 2, 2, 256], F32, tag="ps")
            nc.gpsimd.memset(scratch[:], 0.0)
            for _ in range(13):
                nc.tensor.matmul(
                    scratch_ps[:, 0, :, :], scratch[:, 0, :P], scratch[:],
                    start=True, stop=True,
                )

            def lhsT(w, kh, mh):
                return w_sb[:, w, kh, mh * P:(mh + 1) * P]

            x_tiles = {}

            def prefetch(ck):
                if ck >= NCHUNK:
                    return
                xt = data.tile([P, 2, NPL, CHUNKS[ck]], F16, name="x", tag="x")
                x_tiles[ck] = xt
                if ck == 0:
                    # xa/xv planes first: the dl/vd matmuls need only these,
                    # so chunk-0 compute starts before the x_d planes land
                    nc.sync.dma_start(xt[:, :, 9:, :], dram_ck(xin_hp, ck, 9))
                    nc.sync.dma_start(xt[:, :, :9, :], dram_ck(xin_hp, ck, 0, 9))
                else:
                    nc.sync.dma_start(xt[:], dram_ck(xin_hp, ck))

            # 3-chunk in-DMA lookahead keeps the sync HWDGE ring free of
            # head-of-line blocking; in/out transfers alternate so HBM reads
            # and writes overlap in the SDMA engines
            for ck in range(3):
                prefetch(ck)

            for ck in range(NCHUNK):
                TCH = CHUNKS[ck]
                x_sb = x_tiles.pop(ck)
                o_sb = outp.tile([P, 2, NPL, TCH], F16, name="o", tag="o")

                # eps-contract and trace of raw x_d
                y_sb = tmp.tile([P, 2, 3, TCH], F16, tag="y")
                tr_sb = tmp.tile([P, 2, TCH], F16, tag="tr")
                for k, (s1, s2) in enumerate(Y_PAIRS):
                    nc.vector.tensor_sub(
                        y_sb[:, :, k, :], x_sb[:, :, s1, :], x_sb[:, :, s2, :]
                    )
                nc.vector.tensor_add(tr_sb[:], x_sb[:, :, 0, :], x_sb[:, :, 4, :])
                nc.vector.tensor_add(tr_sb[:], tr_sb[:], x_sb[:, :, 8, :])

                def mm_group(ps, wslot, rhs, npl=2):
                    """ps[:, mh, :npl, :] = W[wslot].T @ rhs (npl planes as
                    one N=npl*TCH matmul), both output-channel halves; each
                    mh half is exactly one PSUM bank."""
                    for mh in range(2):
                        for kh in range(2):
                            nc.tensor.matmul(
                                ps[:, mh, :npl, :], lhsT(wslot, kh, mh), rhs(kh),
                                start=(kh == 0), stop=(kh == 1),
                            )

                # shared products, computed once per chunk (vd pairs planes)
                dl_sb = tmp.tile([P, 2, TCH], F16, tag="dl")
                ps = psum.tile([P, 2, 2, TCH], F32, tag="ps")
                mm_group(ps, 1, lambda kh: x_sb[:, kh, PL_XA, :], npl=1)
                nc.scalar.copy(dl_sb[:], ps[:, :, 0, :])

                vd_sb = tmp.tile([P, 2, 3, TCH], F16, tag="vd")
                ps = psum.tile([P, 2, 2, TCH], F32, tag="ps")
                mm_group(ps, 2, lambda kh: x_sb[:, kh, PL_XV:PL_XV + 2, :])
                nc.scalar.copy(vd_sb[:, :, 0:2, :], ps[:])
                ps = psum.tile([P, 2, 2, TCH], F32, tag="ps")
                mm_group(ps, 2, lambda kh: x_sb[:, kh, PL_XV + 2, :], npl=1)
                nc.scalar.copy(vd_sb[:, :, 2, :], ps[:, :, 0, :])

                # x_d planes, output order SPERM (so rhs planes are the
                # consecutive raw planes 0..8 -> contiguous N=512 pairs)
                def combine(s, ps_plane):
                    if s in DIAG:
                        nc.vector.tensor_add(o_sb[:, :, s, :], ps_plane, dl_sb[:])
                    else:
                        istar, sign = VD_MAP[s]
                        op = nc.vector.tensor_add if sign > 0 else nc.vector.tensor_sub
                        op(o_sb[:, :, s, :], ps_plane, vd_sb[:, :, istar, :])

                last = ck == NCHUNK - 1
                # last chunk: pair order puts planes 0-3 first so the out
                # pieces can stream while the remaining planes compute
                for q in ([0, 1, 3, 2] if last else range(4)):
                    ps = psum.tile([P, 2, 2, TCH], F32, tag="ps")
                    mm_group(ps, 0, lambda kh, q=q: x_sb[:, kh, 2 * q:2 * q + 2, :])
                    combine(SPERM[2 * q], ps[:, :, 0, :])
                    combine(SPERM[2 * q + 1], ps[:, :, 1, :])

                ps = psum.tile([P, 2, 2, TCH], F32, tag="ps")
                mm_group(ps, 0, lambda kh: x_sb[:, kh, 8, :], npl=1)
                combine(8, ps[:, :, 0, :])


                # x_v planes: delta_v = W_cd @ y (paired)
                ps = psum.tile([P, 2, 2, TCH], F32, tag="ps")
                mm_group(ps, 3, lambda kh: y_sb[:, kh, 0:2, :])
                nc.scalar.copy(o_sb[:, :, PL_XV:PL_XV + 2, :], ps[:])
                ps = psum.tile([P, 2, 2, TCH], F32, tag="ps")
                mm_group(ps, 3, lambda kh: y_sb[:, kh, 2, :], npl=1)
                nc.scalar.copy(o_sb[:, :, PL_XV + 2, :], ps[:, :, 0, :])

                # x_a plane: delta_a = W_ct @ tr
                ps = psum.tile([P, 2, 2, TCH], F32, tag="ps")
                mm_group(ps, 4, lambda kh: tr_sb[:, kh, :], npl=1)
                nc.scalar.copy(o_sb[:, :, PL_XA, :], ps[:, :, 0, :])

                if ck == NCHUNK - 1:
                    nc.sync.dma_start(dram_ck(xout_hp, ck), o_sb[:])
                else:
                    nc.sync.dma_start(
                        dram_ck(xout_hp, ck, 0, OUT_SPLIT), o_sb[:, :, :OUT_SPLIT, :]
                    )
                    prefetch(ck + 3)
                    nc.sync.dma_start(
                        dram_ck(xout_hp, ck, OUT_SPLIT), o_sb[:, :, OUT_SPLIT:, :]
                    )
    nc.compile()
    return nc


def _get_nc():
    if "nc" not in _CACHE:
        _CACHE["nc"] = _build()
    return _CACHE["nc"]


def kernel(x_a, x_v, x_d, W_ttrans, W_ttrace, W_tdelta, W_vd, W_dv, **_ignored):
    x_a = np.asarray(x_a, dtype=np.float32)
    x_v = np.asarray(x_v, dtype=np.float32)
    x_d = np.asarray(x_d, dtype=np.float32)
    W_ttrans = np.asarray(W_ttrans, dtype=np.float32)
    W_ttrace = np.asarray(W_ttrace, dtype=np.float32)
    W_tdelta = np.asarray(W_tdelta, dtype=np.float32)
    W_vd = np.asarray(W_vd, dtype=np.float32)
    W_dv = np.asarray(W_dv, dtype=np.float32)

    eye = np.eye(C, dtype=np.float32)
    W_ct = (W_ttrace @ (eye + W_ttrans)).astype(np.float32)
    W_cd = (W_dv @ (eye - W_ttrans)).astype(np.float32)
    wts = np.ascontiguousarray(
        np.stack([W_ttrans.T, W_tdelta.T, W_vd.T, W_cd.T, W_ct.T]).astype(np.float16)
    )

    # host reshard: plane-major [core, C, 13, T] then repack per chunk into
    # the flat per-chunk-contiguous device layout
    big = np.empty((NCORES, C, NPL, T), dtype=np.float16)
    big[:, :, 0:9, :] = x_d.reshape(NCORES, T, C, 9).transpose(0, 2, 3, 1)
    big[:, :, 9:12, :] = x_v.reshape(NCORES, T, C, 3).transpose(0, 2, 3, 1)
    big[:, :, 12, :] = x_a.reshape(NCORES, T, C).transpose(0, 2, 1)
    xin = np.empty((NCORES, C, NPL * T), dtype=np.float16)
    for ck, tch in enumerate(CHUNKS):
        off, toff = NPL * OFFS[ck], OFFS[ck]
        xin[:, :, off:off + NPL * tch] = (
            big[:, :, :, toff:toff + tch].reshape(NCORES, C, NPL * tch)
        )

    in_maps = [{"xin": xin[c], "wts": wts} for c in range(NCORES)]

    nc = _get_nc()
    res = run_bass_kernel_spmd(nc, in_maps, core_ids=list(range(NCORES)))

    xout = np.stack([res.results[c]["xout"] for c in range(NCORES)])
    # device returns fp16 deltas; unpack chunks, residual add in exact fp32
    dbig = np.empty((NCORES, C, NPL, T), dtype=np.float16)
    for ck, tch in enumerate(CHUNKS):
        off, toff = NPL * OFFS[ck], OFFS[ck]
        dbig[:, :, :, toff:toff + tch] = (
            xout[:, :, off:off + NPL * tch].reshape(NCORES, C, NPL, tch)
        )
    # [core, C, plane, t] -> [core, t, C, plane]
    delta = dbig.transpose(0, 3, 1, 2).astype(np.float32)
    x_d_out = x_d + np.ascontiguousarray(delta[..., 0:9]).reshape(B, N, C, 3, 3)
    x_v_out = x_v + np.ascontiguousarray(delta[..., 9:12]).reshape(B, N, C, 3)
    x_a_out = x_a + np.ascontiguousarray(delta[..., 12]).reshape(B, N, C)
    return (x_a_out, x_v_out, x_d_out)


# revision 40
# speedup vs baseline: 1.1363x; 1.1363x over previous
"""Trainium2 Bass kernel for nn_AVDFullLinearMix.

Math (folded form, all terms single matmuls over raw inputs):
  x_d_out = x_d + W_ttrans @ x_d^T(spatial) + (W_tdelta @ x_a) * eye3
            + eps_expand(W_vd @ x_v)
  x_a_out = x_a + W_ct @ trace(x_d),        W_ct = W_ttrace @ (I + W_ttrans)
  x_v_out = x_v + W_cd @ eps_contract(x_d), W_cd = W_dv @ (I - W_ttrans)
(the TensDelta term never reaches the eps-contract since eps[i,i,k] = 0,
 and trace/eps-contract commute with the TensTrans spatial transpose up to
 identity/negation, so both weight chains fold on the host.)

The device computes and returns only the DELTAS in fp16; the exact-fp32
residual add (x + delta) happens on the host. This halves both stream
directions (inputs also stream fp16 since every consumer goes through the
fp16/fp22 matmul datapath anyway).

Sharding: data-parallel over 8 cores (2048 tokens each); weights replicated.
Device layout is channel-major and chunk-packed: one input tensor
[256, chunk, 13, tok] per core holding (9 x_d planes, 3 x_v planes, 1 x_a).
The sync HWDGE ring carries everything with a 3-chunk in-DMA lookahead so
the FIFO never head-of-line blocks, and in/out transfers alternate so HBM
reads and writes overlap. The shared delta (W_tdelta @ x_a) and the three
vd products (W_vd @ x_v) are computed once per chunk and reused across the
diagonal / off-diagonal planes via the DVE combine (add/sub handles the
Levi-Civita sign); pure PSUM->SBUF copies ride the otherwise-idle Scalar
engine so VectorE only does the y/tr builds and the 9 ttrans combines.
"""

import numpy as np

import concourse.bass as bass  # noqa: F401  (registers handle types)
import concourse.mybir as mybir
import concourse.tile as tile
from concourse import bacc
from concourse.bass_utils import run_bass_kernel_spmd

NCORES = 8
P = 128          # partitions
C = 256          # channels
B, N = 16, 1024
T = (B * N) // NCORES   # tokens per core = 2048
CHUNKS = [128] + [256] * 7 + [128]  # small chunks at BOTH ends: short head fill
OFFS = [0]                         # and a short compute tail after the last input
for _tch in CHUNKS[:-1]:
    OFFS.append(OFFS[-1] + _tch)
NCHUNK = len(CHUNKS)
NPL = 13                # planes per chunk: 9 x_d, 3 x_v, 1 x_a
PL_XV = 9
PL_XA = 12
OUT_SPLIT = 7           # out-DMA part A = planes [0:7), part B = [7:13)

F32 = mybir.dt.float32
F16 = mybir.dt.float16

# flat spatial index s = i*3 + j
SPERM = [0, 3, 6, 1, 4, 7, 2, 5, 8]    # s -> transposed flat (j*3+i)
DIAG = (0, 4, 8)
# off-diag s=(k,j): x_d_out[...,k,j] += eps[i*,j,k] * (W_vd @ x_v[..,i*])
# VD_MAP: s -> (i*, sign)
VD_MAP = {1: (2, -1), 2: (1, +1), 3: (2, +1), 5: (0, -1), 6: (1, -1), 7: (0, +1)}
# eps-contract y[:,k] = x_d[:,s1] - x_d[:,s2]
Y_PAIRS = [(5, 7), (6, 2), (1, 3)]
# weight slots (lhsT layout W.T): 0=ttrans 1=tdelta 2=vd 3=cd 4=ct
NW = 5

_CACHE: dict = {}


def _build():
    nc = bacc.Bacc(None, target_bir_lowering=False)
    xin = nc.dram_tensor("xin", [C, NPL * T], F16, kind="ExternalInput")
    wts = nc.dram_tensor("wts", [NW, C, C], F16, kind="ExternalInput")
    xout = nc.dram_tensor("xout", [C, NPL * T], F16, kind="ExternalOutput")

    # channel-split dram views: c = h*128 + p
    xin_hp = xin.rearrange("(h p) f -> p h f", p=P)
    xout_hp = xout.rearrange("(h p) f -> p h f", p=P)

    def dram_ck(base, ck, lo=0, hi=NPL):
        tch = CHUNKS[ck]
        off = NPL * OFFS[ck]
        v = base[:, :, off + lo * tch: off + hi * tch]
        return v.rearrange("p h (s t) -> p h s t", t=tch)
    wts_r = wts.rearrange("w (kh p) o -> p w kh o", p=P)

    with tile.TileContext(nc) as tc:
        with (
            tc.tile_pool(name="wpool", bufs=1) as wpool,
            tc.tile_pool(name="data", bufs=3) as data,
            tc.tile_pool(name="outp", bufs=3) as outp,
            tc.tile_pool(name="tmp", bufs=2) as tmp,
            tc.tile_pool(name="psum", bufs=4, space="PSUM") as psum,
        ):
            w_sb = wpool.tile([P, NW, 2, C], F16)

            # HAM warm-up: discarded matmuls while the first input DMA is in
            # flight, so chunk 0 runs at 2.4 GHz instead of 1.2
            scratch = wpool.tile([P, 2, 256], F16)
            scratch_ps = psum.tile([P, 2, 2, 256], F32, tag="ps")
            nc.gpsimd.memset(scratch[:], 0.0)
            for _ in range(10):
                nc.tensor.matmul(
                    scratch_ps[:, 0, :, :], scratch[:, 0, :P], scratch[:],
                    start=True, stop=True,
                )

            def lhsT(w, kh, mh):
                return w_sb[:, w, kh, mh * P:(mh + 1) * P]

            x_tiles = {}

            def prefetch(ck):
                if ck >= NCHUNK:
                    return
                xt = data.tile([P, 2, NPL, CHUNKS[ck]], F16, name="x", tag="x")
                x_tiles[ck] = xt
                nc.sync.dma_start(xt[:], dram_ck(xin_hp, ck))

            # head order: the dl/vd matmuls need only w slots 1-2 and chunk
            # 0's xa/xv planes, so those two transfers go first; the rest of
            # the weights and chunk-0's x_d planes follow, then the lookahead
            x0 = data.tile([P, 2, NPL, CHUNKS[0]], F16, name="x", tag="x")
            x_tiles[0] = x0
            nc.sync.dma_start(w_sb[:, 1:3], wts_r[:, 1:3])
            nc.sync.dma_start(x0[:, :, 9:, :], dram_ck(xin_hp, 0, 9))
            nc.sync.dma_start(w_sb[:, 0], wts_r[:, 0])
            nc.sync.dma_start(x0[:, :, :9, :], dram_ck(xin_hp, 0, 0, 9))
            nc.sync.dma_start(w_sb[:, 3:], wts_r[:, 3:])
            prefetch(1)
            prefetch(2)

            for ck in range(NCHUNK):
                TCH = CHUNKS[ck]
                x_sb = x_tiles.pop(ck)
                o_sb = outp.tile([P, 2, NPL, TCH], F16, name="o", tag="o")

                # eps-contract and trace of raw x_d
                y_sb = tmp.tile([P, 2, 3, TCH], F16, tag="y")
                tr_sb = tmp.tile([P, 2, TCH], F16, tag="tr")
                for k, (s1, s2) in enumerate(Y_PAIRS):
                    nc.vector.tensor_sub(
                        y_sb[:, :, k, :], x_sb[:, :, s1, :], x_sb[:, :, s2, :]
                    )
                nc.vector.tensor_add(tr_sb[:], x_sb[:, :, 0, :], x_sb[:, :, 4, :])
                nc.vector.tensor_add(tr_sb[:], tr_sb[:], x_sb[:, :, 8, :])

                def mm_group(ps, wslot, rhs, npl=2):
                    """ps[:, mh, :npl, :] = W[wslot].T @ rhs (npl planes as
                    one N=npl*TCH matmul), both output-channel halves; each
                    mh half is exactly one PSUM bank."""
                    for mh in range(2):
                        for kh in range(2):
                            nc.tensor.matmul(
                                ps[:, mh, :npl, :], lhsT(wslot, kh, mh), rhs(kh),
                                start=(kh == 0), stop=(kh == 1),
                            )

                # shared products, computed once per chunk (vd pairs planes)
                dl_sb = tmp.tile([P, 2, TCH], F16, tag="dl")
                ps = psum.tile([P, 2, 2, TCH], F32, tag="ps")
                mm_group(ps, 1, lambda kh: x_sb[:, kh, PL_XA, :], npl=1)
                nc.scalar.copy(dl_sb[:], ps[:, :, 0, :])

                vd_sb = tmp.tile([P, 2, 3, TCH], F16, tag="vd")
                ps = psum.tile([P, 2, 2, TCH], F32, tag="ps")
                mm_group(ps, 2, lambda kh: x_sb[:, kh, PL_XV:PL_XV + 2, :])
                nc.scalar.copy(vd_sb[:, :, 0:2, :], ps[:])
                ps = psum.tile([P, 2, 2, TCH], F32, tag="ps")
                mm_group(ps, 2, lambda kh: x_sb[:, kh, PL_XV + 2, :], npl=1)
                nc.scalar.copy(vd_sb[:, :, 2, :], ps[:, :, 0, :])

                # x_d planes, output order SPERM (so rhs planes are the
                # consecutive raw planes 0..8 -> contiguous N=512 pairs)
                def combine(s, ps_plane):
                    if s in DIAG:
                        nc.vector.tensor_add(o_sb[:, :, s, :], ps_plane, dl_sb[:])
                    else:
                        istar, sign = VD_MAP[s]
                        op = nc.vector.tensor_add if sign > 0 else nc.vector.tensor_sub
                        op(o_sb[:, :, s, :], ps_plane, vd_sb[:, :, istar, :])

                last = ck == NCHUNK - 1
                # last chunk: pair order puts planes 0-3 first so the out
                # pieces can stream while the remaining planes compute
                for q in ([0, 1, 3, 2] if last else range(4)):
                    ps = psum.tile([P, 2, 2, TCH], F32, tag="ps")
                    mm_group(ps, 0, lambda kh, q=q: x_sb[:, kh, 2 * q:2 * q + 2, :])
                    combine(SPERM[2 * q], ps[:, :, 0, :])
                    combine(SPERM[2 * q + 1], ps[:, :, 1, :])

                ps = psum.tile([P, 2, 2, TCH], F32, tag="ps")
                mm_group(ps, 0, lambda kh: x_sb[:, kh, 8, :], npl=1)
                combine(8, ps[:, :, 0, :])


                # x_v planes: delta_v = W_cd @ y (paired)
                ps = psum.tile([P, 2, 2, TCH], F32, tag="ps")
                mm_group(ps, 3, lambda kh: y_sb[:, kh, 0:2, :])
                nc.scalar.copy(o_sb[:, :, PL_XV:PL_XV + 2, :], ps[:])
                ps = psum.tile([P, 2, 2, TCH], F32, tag="ps")
                mm_group(ps, 3, lambda kh: y_sb[:, kh, 2, :], npl=1)
                nc.scalar.copy(o_sb[:, :, PL_XV + 2, :], ps[:, :, 0, :])

                # x_a plane: delta_a = W_ct @ tr
                ps = psum.tile([P, 2, 2, TCH], F32, tag="ps")
                mm_group(ps, 4, lambda kh: tr_sb[:, kh, :], npl=1)
                nc.scalar.copy(o_sb[:, :, PL_XA, :], ps[:, :, 0, :])

                if ck == NCHUNK - 1:
                    nc.sync.dma_start(dram_ck(xout_hp, ck), o_sb[:])
                else:
                    nc.sync.dma_start(
                        dram_ck(xout_hp, ck, 0, OUT_SPLIT), o_sb[:, :, :OUT_SPLIT, :]
                    )
                    prefetch(ck + 3)
                    nc.sync.dma_start(
                        dram_ck(xout_hp, ck, OUT_SPLIT), o_sb[:, :, OUT_SPLIT:, :]
                    )
    nc.compile()
    return nc


def _get_nc():
    if "nc" not in _CACHE:
        _CACHE["nc"] = _build()
    return _CACHE["nc"]


def kernel(x_a, x_v, x_d, W_ttrans, W_ttrace, W_tdelta, W_vd, W_dv, **_ignored):
    x_a = np.asarray(x_a, dtype=np.float32)
    x_v = np.asarray(x_v, dtype=np.float32)
    x_d = np.asarray(x_d, dtype=np.float32)
    W_ttrans = np.asarray(W_ttrans, dtype=np.float32)
    W_ttrace = np.asarray(W_ttrace, dtype=np.float32)
    W_tdelta = np.asarray(W_tdelta, dtype=np.float32)
    W_vd = np.asarray(W_vd, dtype=np.float32)
    W_dv = np.asarray(W_dv, dtype=np.float32)

    eye = np.eye(C, dtype=np.float32)
    W_ct = (W_ttrace @ (eye + W_ttrans)).astype(np.float32)
    W_cd = (W_dv @ (eye - W_ttrans)).astype(np.float32)
    wts = np.ascontiguousarray(
        np.stack([W_ttrans.T, W_tdelta.T, W_vd.T, W_cd.T, W_ct.T]).astype(np.float16)
    )

    # host reshard: plane-major [core, C, 13, T] then repack per chunk into
    # the flat per-chunk-contiguous device layout
    big = np.empty((NCORES, C, NPL, T), dtype=np.float16)
    big[:, :, 0:9, :] = x_d.reshape(NCORES, T, C, 9).transpose(0, 2, 3, 1)
    big[:, :, 9:12, :] = x_v.reshape(NCORES, T, C, 3).transpose(0, 2, 3, 1)
    big[:, :, 12, :] = x_a.reshape(NCORES, T, C).transpose(0, 2, 1)
    xin = np.empty((NCORES, C, NPL * T), dtype=np.float16)
    for ck, tch in enumerate(CHUNKS):
        off, toff = NPL * OFFS[ck], OFFS[ck]
        xin[:, :, off:off + NPL * tch] = (
            big[:, :, :, toff:toff + tch].reshape(NCORES, C, NPL * tch)
        )

    in_maps = [{"xin": xin[c], "wts": wts} for c in range(NCORES)]

    nc = _get_nc()
    res = run_bass_kernel_spmd(nc, in_maps, core_ids=list(range(NCORES)))

    xout = np.stack([res.results[c]["xout"] for c in range(NCORES)])
    # device returns fp16 deltas; unpack chunks, residual add in exact fp32
    dbig = np.empty((NCORES, C, NPL, T), dtype=np.float16)
    for ck, tch in enumerate(CHUNKS):
        off, toff = NPL * OFFS[ck], OFFS[ck]
        dbig[:, :, :, toff:toff + tch] = (
            xout[:, :, off:off + NPL * tch].reshape(NCORES, C, NPL, tch)
        )
    # [core, C, plane, t] -> [core, t, C, plane]
    delta = dbig.transpose(0, 3, 1, 2).astype(np.float32)
    x_d_out = x_d + np.ascontiguousarray(delta[..., 0:9]).reshape(B, N, C, 3, 3)
    x_v_out = x_v + np.ascontiguousarray(delta[..., 9:12]).reshape(B, N, C, 3)
    x_a_out = x_a + np.ascontiguousarray(delta[..., 12]).reshape(B, N, C)
    return (x_a_out, x_v_out, x_d_out)


# revision 41
# speedup vs baseline: 1.1850x; 1.0428x over previous
"""Trainium2 Bass kernel for nn_AVDFullLinearMix.

Math (folded form, all terms single matmuls over raw inputs):
  x_d_out = x_d + W_ttrans @ x_d^T(spatial) + (W_tdelta @ x_a) * eye3
            + eps_expand(W_vd @ x_v)
  x_a_out = x_a + W_ct @ trace(x_d),        W_ct = W_ttrace @ (I + W_ttrans)
  x_v_out = x_v + W_cd @ eps_contract(x_d), W_cd = W_dv @ (I - W_ttrans)
(the TensDelta term never reaches the eps-contract since eps[i,i,k] = 0,
 and trace/eps-contract commute with the TensTrans spatial transpose up to
 identity/negation, so both weight chains fold on the host.)

The device computes and returns only the DELTAS in fp16; the exact-fp32
residual add (x + delta) happens on the host. This halves both stream
directions (inputs also stream fp16 since every consumer goes through the
fp16/fp22 matmul datapath anyway).

Sharding: data-parallel over 8 cores (2048 tokens each); weights replicated.
Device layout is channel-major and chunk-packed: one input tensor
[256, chunk, 13, tok] per core holding (9 x_d planes, 3 x_v planes, 1 x_a).
The sync HWDGE ring carries everything with a 3-chunk in-DMA lookahead so
the FIFO never head-of-line blocks, and in/out transfers alternate so HBM
reads and writes overlap. The shared delta (W_tdelta @ x_a) and the three
vd products (W_vd @ x_v) are computed once per chunk and reused across the
diagonal / off-diagonal planes via the DVE combine (add/sub handles the
Levi-Civita sign); pure PSUM->SBUF copies ride the otherwise-idle Scalar
engine so VectorE only does the y/tr builds and the 9 ttrans combines.
"""

import numpy as np

import concourse.bass as bass  # noqa: F401  (registers handle types)
import concourse.mybir as mybir
import concourse.tile as tile
from concourse import bacc
from concourse.bass_utils import run_bass_kernel_spmd

NCORES = 8
P = 128          # partitions
C = 256          # channels
B, N = 16, 1024
T = (B * N) // NCORES   # tokens per core = 2048
CHUNKS = [128] + [256] * 7 + [128]  # small chunks at BOTH ends: short head fill
OFFS = [0]                         # and a short compute tail after the last input
for _tch in CHUNKS[:-1]:
    OFFS.append(OFFS[-1] + _tch)
NCHUNK = len(CHUNKS)
NPL = 13                # planes per chunk: 9 x_d, 3 x_v, 1 x_a
PL_XV = 9
PL_XA = 12
OUT_SPLIT = 7           # out-DMA part A = planes [0:7), part B = [7:13)

F32 = mybir.dt.float32
F16 = mybir.dt.float16

# flat spatial index s = i*3 + j
SPERM = [0, 3, 6, 1, 4, 7, 2, 5, 8]    # s -> transposed flat (j*3+i)
DIAG = (0, 4, 8)
# off-diag s=(k,j): x_d_out[...,k,j] += eps[i*,j,k] * (W_vd @ x_v[..,i*])
# VD_MAP: s -> (i*, sign)
VD_MAP = {1: (2, -1), 2: (1, +1), 3: (2, +1), 5: (0, -1), 6: (1, -1), 7: (0, +1)}
# eps-contract y[:,k] = x_d[:,s1] - x_d[:,s2]
Y_PAIRS = [(5, 7), (6, 2), (1, 3)]
# weight slots (lhsT layout W.T): 0=ttrans 1=tdelta 2=vd 3=cd 4=ct
NW = 5

_CACHE: dict = {}


def _build():
    nc = bacc.Bacc(None, target_bir_lowering=False)
    xin = nc.dram_tensor("xin", [C, NPL * T], F16, kind="ExternalInput")
    wts = nc.dram_tensor("wts", [NW, C, C], F16, kind="ExternalInput")
    xout = nc.dram_tensor("xout", [C, NPL * T], F16, kind="ExternalOutput")

    # channel-split dram views: c = h*128 + p
    xin_hp = xin.rearrange("(h p) f -> p h f", p=P)
    xout_hp = xout.rearrange("(h p) f -> p h f", p=P)

    def dram_ck(base, ck, lo=0, hi=NPL):
        tch = CHUNKS[ck]
        off = NPL * OFFS[ck]
        v = base[:, :, off + lo * tch: off + hi * tch]
        return v.rearrange("p h (s t) -> p h s t", t=tch)
    wts_r = wts.rearrange("w (kh p) o -> p w kh o", p=P)

    with tile.TileContext(nc) as tc:
        with (
            tc.tile_pool(name="wpool", bufs=1) as wpool,
            tc.tile_pool(name="data", bufs=3) as data,
            tc.tile_pool(name="outp", bufs=3) as outp,
            tc.tile_pool(name="tmp", bufs=2) as tmp,
            tc.tile_pool(name="psum", bufs=4, space="PSUM") as psum,
        ):
            w_sb = wpool.tile([P, NW, 2, C], F16)

            # HAM warm-up: discarded matmuls while the first input DMA is in
            # flight, so chunk 0 runs at 2.4 GHz instead of 1.2
            scratch = wpool.tile([P, 2, 256], F16)
            scratch_ps = psum.tile([P, 2, 2, 256], F32, tag="ps")
            nc.gpsimd.memset(scratch[:], 0.0)
            for _ in range(10):
                nc.tensor.matmul(
                    scratch_ps[:, 0, :, :], scratch[:, 0, :P], scratch[:],
                    start=True, stop=True,
                )

            def lhsT(w, kh, mh):
                return w_sb[:, w, kh, mh * P:(mh + 1) * P]

            x_tiles = {}

            def prefetch(ck):
                if ck >= NCHUNK:
                    return
                xt = data.tile([P, 2, NPL, CHUNKS[ck]], F16, name="x", tag="x")
                x_tiles[ck] = xt
                nc.sync.dma_start(xt[:], dram_ck(xin_hp, ck))

            # head order: the dl/vd matmuls need only w slots 1-2 and chunk
            # 0's xa/xv planes, so those two transfers go first; the rest of
            # the weights and chunk-0's x_d planes follow, then the lookahead
            x0 = data.tile([P, 2, NPL, CHUNKS[0]], F16, name="x", tag="x")
            x_tiles[0] = x0
            nc.sync.dma_start(w_sb[:, 1:3], wts_r[:, 1:3])
            nc.sync.dma_start(x0[:, :, 9:, :], dram_ck(xin_hp, 0, 9))
            nc.sync.dma_start(w_sb[:, 0], wts_r[:, 0])
            nc.sync.dma_start(x0[:, :, :9, :], dram_ck(xin_hp, 0, 0, 9))
            nc.sync.dma_start(w_sb[:, 3:], wts_r[:, 3:])
            prefetch(1)
            prefetch(2)

            for ck in range(NCHUNK):
                TCH = CHUNKS[ck]
                x_sb = x_tiles.pop(ck)
                o_sb = outp.tile([P, 2, NPL, TCH], F16, name="o", tag="o")

                # eps-contract and trace of raw x_d
                y_sb = tmp.tile([P, 2, 3, TCH], F16, tag="y")
                tr_sb = tmp.tile([P, 2, TCH], F16, tag="tr")
                for k, (s1, s2) in enumerate(Y_PAIRS):
                    nc.vector.tensor_sub(
                        y_sb[:, :, k, :], x_sb[:, :, s1, :], x_sb[:, :, s2, :]
                    )
                nc.vector.tensor_add(tr_sb[:], x_sb[:, :, 0, :], x_sb[:, :, 4, :])
                nc.vector.tensor_add(tr_sb[:], tr_sb[:], x_sb[:, :, 8, :])

                def mm_group(ps, wslot, rhs, npl=2):
                    """ps[:, mh, :npl, :] = W[wslot].T @ rhs (npl planes as
                    one N=npl*TCH matmul), both output-channel halves; each
                    mh half is exactly one PSUM bank."""
                    for mh in range(2):
                        for kh in range(2):
                            nc.tensor.matmul(
                                ps[:, mh, :npl, :], lhsT(wslot, kh, mh), rhs(kh),
                                start=(kh == 0), stop=(kh == 1),
                            )

                # shared products, computed once per chunk (vd pairs planes)
                dl_sb = tmp.tile([P, 2, TCH], F16, tag="dl")
                ps = psum.tile([P, 2, 2, TCH], F32, tag="ps")
                mm_group(ps, 1, lambda kh: x_sb[:, kh, PL_XA, :], npl=1)
                nc.scalar.copy(dl_sb[:], ps[:, :, 0, :])

                vd_sb = tmp.tile([P, 2, 3, TCH], F16, tag="vd")
                ps = psum.tile([P, 2, 2, TCH], F32, tag="ps")
                mm_group(ps, 2, lambda kh: x_sb[:, kh, PL_XV:PL_XV + 2, :])
                nc.scalar.copy(vd_sb[:, :, 0:2, :], ps[:])
                ps = psum.tile([P, 2, 2, TCH], F32, tag="ps")
                mm_group(ps, 2, lambda kh: x_sb[:, kh, PL_XV + 2, :], npl=1)
                nc.scalar.copy(vd_sb[:, :, 2, :], ps[:, :, 0, :])

                # x_d planes, output order SPERM (so rhs planes are the
                # consecutive raw planes 0..8 -> contiguous N=512 pairs)
                def combine(s, ps_plane):
                    if s in DIAG:
                        nc.vector.tensor_add(o_sb[:, :, s, :], ps_plane, dl_sb[:])
                    else:
                        istar, sign = VD_MAP[s]
                        op = nc.vector.tensor_add if sign > 0 else nc.vector.tensor_sub
                        op(o_sb[:, :, s, :], ps_plane, vd_sb[:, :, istar, :])

                last = ck == NCHUNK - 1
                # last chunk: pair order puts planes 0-3 first so the out
                # pieces can stream while the remaining planes compute
                for q in ([0, 1, 3, 2] if last else range(4)):
                    ps = psum.tile([P, 2, 2, TCH], F32, tag="ps")
                    mm_group(ps, 0, lambda kh, q=q: x_sb[:, kh, 2 * q:2 * q + 2, :])
                    combine(SPERM[2 * q], ps[:, :, 0, :])
                    combine(SPERM[2 * q + 1], ps[:, :, 1, :])

                ps = psum.tile([P, 2, 2, TCH], F32, tag="ps")
                mm_group(ps, 0, lambda kh: x_sb[:, kh, 8, :], npl=1)
                combine(8, ps[:, :, 0, :])


                # x_v planes: delta_v = W_cd @ y (paired)
                ps = psum.tile([P, 2, 2, TCH], F32, tag="ps")
                mm_group(ps, 3, lambda kh: y_sb[:, kh, 0:2, :])
                nc.scalar.copy(o_sb[:, :, PL_XV:PL_XV + 2, :], ps[:])
                ps = psum.tile([P, 2, 2, TCH], F32, tag="ps")
                mm_group(ps, 3, lambda kh: y_sb[:, kh, 2, :], npl=1)
                nc.scalar.copy(o_sb[:, :, PL_XV + 2, :], ps[:, :, 0, :])

                # x_a plane: delta_a = W_ct @ tr
                ps = psum.tile([P, 2, 2, TCH], F32, tag="ps")
                mm_group(ps, 4, lambda kh: tr_sb[:, kh, :], npl=1)
                nc.scalar.copy(o_sb[:, :, PL_XA, :], ps[:, :, 0, :])

                if ck == NCHUNK - 1:
                    nc.sync.dma_start(dram_ck(xout_hp, ck, 0, 9), o_sb[:, :, :9, :])
                    nc.sync.dma_start(dram_ck(xout_hp, ck, 9), o_sb[:, :, 9:, :])
                else:
                    nc.sync.dma_start(
                        dram_ck(xout_hp, ck, 0, OUT_SPLIT), o_sb[:, :, :OUT_SPLIT, :]
                    )
                    prefetch(ck + 3)
                    nc.sync.dma_start(
                        dram_ck(xout_hp, ck, OUT_SPLIT), o_sb[:, :, OUT_SPLIT:, :]
                    )
    nc.compile()
    return nc


def _get_nc():
    if "nc" not in _CACHE:
        _CACHE["nc"] = _build()
    return _CACHE["nc"]


def kernel(x_a, x_v, x_d, W_ttrans, W_ttrace, W_tdelta, W_vd, W_dv, **_ignored):
    x_a = np.asarray(x_a, dtype=np.float32)
    x_v = np.asarray(x_v, dtype=np.float32)
    x_d = np.asarray(x_d, dtype=np.float32)
    W_ttrans = np.asarray(W_ttrans, dtype=np.float32)
    W_ttrace = np.asarray(W_ttrace, dtype=np.float32)
    W_tdelta = np.asarray(W_tdelta, dtype=np.float32)
    W_vd = np.asarray(W_vd, dtype=np.float32)
    W_dv = np.asarray(W_dv, dtype=np.float32)

    eye = np.eye(C, dtype=np.float32)
    W_ct = (W_ttrace @ (eye + W_ttrans)).astype(np.float32)
    W_cd = (W_dv @ (eye - W_ttrans)).astype(np.float32)
    wts = np.ascontiguousarray(
        np.stack([W_ttrans.T, W_tdelta.T, W_vd.T, W_cd.T, W_ct.T]).astype(np.float16)
    )

    # host reshard: plane-major [core, C, 13, T] then repack per chunk into
    # the flat per-chunk-contiguous device layout
    big = np.empty((NCORES, C, NPL, T), dtype=np.float16)
    big[:, :, 0:9, :] = x_d.reshape(NCORES, T, C, 9).transpose(0, 2, 3, 1)
    big[:, :, 9:12, :] = x_v.reshape(NCORES, T, C, 3).transpose(0, 2, 3, 1)
    big[:, :, 12, :] = x_a.reshape(NCORES, T, C).transpose(0, 2, 1)
    xin = np.empty((NCORES, C, NPL * T), dtype=np.float16)
    for ck, tch in enumerate(CHUNKS):
        off, toff = NPL * OFFS[ck], OFFS[ck]
        xin[:, :, off:off + NPL * tch] = (
            big[:, :, :, toff:toff + tch].reshape(NCORES, C, NPL * tch)
        )

    in_maps = [{"xin": xin[c], "wts": wts} for c in range(NCORES)]

    nc = _get_nc()
    res = run_bass_kernel_spmd(nc, in_maps, core_ids=list(range(NCORES)))

    xout = np.stack([res.results[c]["xout"] for c in range(NCORES)])
    # device returns fp16 deltas; unpack chunks, residual add in exact fp32
    dbig = np.empty((NCORES, C, NPL, T), dtype=np.float16)
    for ck, tch in enumerate(CHUNKS):
        off, toff = NPL * OFFS[ck], OFFS[ck]
        dbig[:, :, :, toff:toff + tch] = (
            xout[:, :, off:off + NPL * tch].reshape(NCORES, C, NPL, tch)
        )
    # [core, C, plane, t] -> [core, t, C, plane]
    delta = dbig.transpose(0, 3, 1, 2).astype(np.float32)
    x_d_out = x_d + np.ascontiguousarray(delta[..., 0:9]).reshape(B, N, C, 3, 3)
    x_v_out = x_v + np.ascontiguousarray(delta[..., 9:12]).reshape(B, N, C, 3)
    x_a_out = x_a + np.ascontiguousarray(delta[..., 12]).reshape(B, N, C)
    return (x_a_out, x_v_out, x_d_out)
